# revision 11
# baseline (speedup 1.0000x reference)
"""FlexibleThresholdedLoss on 8 Trainium2 NeuronCores.

Strategy (pure data parallel over the batch dim): each core gets 4 of the
32 images of both inputs, viewed as [128, 24576] f32 per tensor.

Phase A (streams 24 MiB f32 from HBM per core; DMA is the binding
resource at ~340-420 GB/s into SBUF):
  - Per 4096-col block: one 2 MiB DMA for the a-chunk, one for the
    b-chunk, into a deep stage pool that keeps the SDMA queue fed.
  - d_b = a_b - b_b on GPSIMD (slow but otherwise idle; frees DVE).
  - u_b = |d_b| on ACT (fp16, per-block resident tiles - block granular
    so next iteration's writes don't serialize against phase B reads).
  - q_b = d_b * d_b on DVE tensor_tensor (fp16, transient scratch).
  - All reductions on PE ones-matmuls into PSUM banks: S_u = sum(u),
    S_q = sum(q).
One AllReduce of (S_u, S_q) -> global means t (mae) and s (mse);
t and r = sqrt(s) are broadcast to all partitions.

Phase B (reads only the resident u blocks; u-domain throughout):
  - m1 = max(u, t)        -> PE sum = M1
  - c1 = [u >= t]         -> PE sum = C1
  - c2 = [u >= r]         -> PE sum = C2
  - mx = max(u, r); qm = Square(mx) on ACT -> PE sum = M2
    (max(u,r)^2 == max(d^2, s) exactly)
Host closes the algebra:
  s1 = M1 - t*(N - C1)   (masked |d| sum;  mae_thr = s1/C1)
  s2 = M2 - s*(N - C2)   (masked d^2 sum;  mse_thr = s2/C2)
"""

import numpy as np

import concourse.bacc as bacc
import concourse.mybir as mybir
from concourse.bass_utils import run_bass_kernel_spmd
from concourse.tile import TileContext

P = 128
FD = 24576            # per-partition elements per input tensor (per core)
N_CORES = 8
N_TOTAL = 32 * 3 * 512 * 512   # 25_165_824 global element count

BLK = 4096            # block: one a-chunk + one b-chunk DMA (2 MiB each)
N_BLK = FD // BLK     # 6
STAGE_BUFS = 6
MM_N = 512            # PE ones-matmul free-dim slice (one PSUM bank)

F32 = mybir.dt.float32
F16 = mybir.dt.float16
ALU = mybir.AluOpType
ACTF = mybir.ActivationFunctionType
AX = mybir.AxisListType

# ablation controls for the timing loop (None = full)
LOOP_PARTS_A = None
LOOP_PARTS_B = None
PARTS_A = frozenset({"dma", "sub", "abs", "sq", "usum"})
PARTS_B = frozenset({"m1", "c1", "c2", "m2"})


def _pe_sum(nc, ones, psum, src, start, stop=False):
    w = src.shape[-1]
    for j in range(0, w, MM_N):
        nc.tensor.matmul(
            psum[:, 0:MM_N],
            ones[:, 0:1],
            src[:, j : j + MM_N],
            start=(start and j == 0),
            stop=(stop and j + MM_N >= w),
        )


def _emit_phase_a(nc, ab_d, ublks, ones, psu, psq, stage_pool, d_pool,
                  scr_pool, parts=PARTS_A):
    for b in range(N_BLK):
        if "dma" not in parts:
            continue
        sta = stage_pool.tile([P, BLK], F32, tag="stage")
        nc.sync.dma_start(sta[:], ab_d[:, 2 * b * BLK : (2 * b + 1) * BLK])
        stb = stage_pool.tile([P, BLK], F32, tag="stage")
        nc.sync.dma_start(stb[:], ab_d[:, (2 * b + 1) * BLK : (2 * b + 2) * BLK])
        if "sub" not in parts:
            continue
        dt_ = d_pool.tile([P, BLK], F16, tag="d")
        nc.gpsimd.tensor_tensor(dt_[:], sta[:], stb[:], op=ALU.subtract)
        if "abs" in parts:
            nc.scalar.activation(ublks[b][:], dt_[:], ACTF.Abs)
            if "usum" in parts:
                _pe_sum(nc, ones, psu, ublks[b][:], start=(b == 0),
                        stop=(b == N_BLK - 1))
        if "sq" in parts:
            q = scr_pool.tile([P, BLK], F16, tag="scr")
            nc.vector.tensor_tensor(q[:], dt_[:], dt_[:], op=ALU.mult)
            _pe_sum(nc, ones, psq, q[:], start=(b == 0),
                    stop=(b == N_BLK - 1))


def _emit_phase_b(nc, ones, ublks, thr, psums, scr_pool, parts=PARTS_B):
    """Thresholded reductions in the u domain. thr[:,0:1]=t, thr[:,1:2]=r."""
    t_ = thr[:, 0:1]
    r_ = thr[:, 1:2]
    specs = [
        ("m1", t_, ALU.max, "pm1", False),
        ("c1", t_, ALU.is_ge, "pc1", False),
        ("c2", r_, ALU.is_ge, "pc2", False),
        ("m2", r_, ALU.max, "pm2", True),
    ]
    for b in range(N_BLK):
        u = ublks[b][:]
        for name, scal, op, pname, square in specs:
            if name not in parts:
                continue
            ot = scr_pool.tile([P, BLK], F16, tag="scr")
            nc.vector.tensor_scalar(ot[:], u, scal, None, op0=op)
            if square:
                qm = scr_pool.tile([P, BLK], F16, tag="scr")
                nc.scalar.activation(qm[:], ot[:], ACTF.Square)
                ot = qm
            _pe_sum(nc, ones, psums[pname], ot[:], start=(b == 0),
                    stop=(b == N_BLK - 1))


def _build_program(loop_n=0):
    nc = bacc.Bacc("TRN2", target_bir_lowering=False)

    # host packs a and b in alternating BLK-col chunks: [P, n_blk, 2, BLK]
    ab_d = nc.declare_dram_parameter("ab", [P, 2 * FD], F32, isOutput=False)
    out_d = nc.declare_dram_parameter("partials", [1, 16], F32, isOutput=True)

    with TileContext(nc) as tc:
        with (
            tc.tile_pool(name="stage", bufs=STAGE_BUFS) as stage_pool,
            tc.tile_pool(name="ures", bufs=1) as ures_pool,
            tc.tile_pool(name="dtr", bufs=3) as d_pool,
            tc.tile_pool(name="scr", bufs=4) as scr_pool,
            tc.tile_pool(name="small", bufs=1) as small_pool,
            tc.tile_pool(name="psum", bufs=1, space="PSUM") as psum_pool,
            tc.tile_pool(name="dram", bufs=1, space="DRAM") as dram_pool,
        ):
            ublks = [
                ures_pool.tile([P, BLK], F16, tag=f"u{b}", name=f"u{b}")
                for b in range(N_BLK)
            ]

            ones = small_pool.tile([P, 1], F16, tag="ones")
            nc.vector.memset(ones[:], 1.0)
            ones32 = small_pool.tile([P, 1], F32, tag="ones32")
            nc.vector.memset(ones32[:], 1.0)

            psu = psum_pool.tile([1, MM_N], F32, tag="psu", name="psu")
            psq = psum_pool.tile([1, MM_N], F32, tag="psq", name="psq")
            psums = {
                n: psum_pool.tile([1, MM_N], F32, tag=n, name=n)
                for n in ("pm1", "pc1", "pc2", "pm2")
            }

            _emit_phase_a(nc, ab_d, ublks, ones, psu, psq, stage_pool,
                          d_pool, scr_pool)

            # ---- core-local scalars, all-reduce across cores ----
            sums2 = small_pool.tile([1, 2], F32, tag="sums2")
            nc.vector.tensor_reduce(sums2[:, 0:1], psu[:, :], axis=AX.X,
                                    op=ALU.add)
            nc.vector.tensor_reduce(sums2[:, 1:2], psq[:, :], axis=AX.X,
                                    op=ALU.add)

            cc_in = dram_pool.tile([1, 2], F32, tag="cc_in")
            cc_out = dram_pool.tile([1, 2], F32, tag="cc_out",
                                    addr_space="Shared")
            nc.sync.dma_start(cc_in[:], sums2[:])
            nc.gpsimd.collective_compute(
                "AllReduce",
                ALU.add,
                replica_groups=[list(range(N_CORES))],
                ins=[cc_in.opt()],
                outs=[cc_out.opt()],
            )
            g = small_pool.tile([1, 2], F32, tag="g")
            nc.sync.dma_start(g[:], cc_out[:])

            # thresholds: t = mae mean, r = sqrt(mse mean), on all partitions
            ts2 = small_pool.tile([1, 2], F32, tag="ts2")
            inv_n = 1.0 / float(N_TOTAL)
            nc.scalar.mul(ts2[:, 0:2], g[:, 0:2], inv_n)
            th = small_pool.tile([1, 2], F32, tag="th")
            nc.scalar.copy(th[:, 0:1], ts2[:, 0:1])
            nc.scalar.activation(th[:, 1:2], ts2[:, 1:2], ACTF.Sqrt)
            thr = small_pool.tile([P, 2], F32, tag="thr")
            nc.gpsimd.partition_broadcast(thr[:], th[:], channels=P)

            _emit_phase_b(nc, ones, ublks, thr, psums, scr_pool)

            # ---- final reductions + output row ----
            outrow = small_pool.tile([1, 16], F32, tag="outrow")
            nc.vector.memset(outrow[:], 0.0)
            nc.scalar.copy(outrow[:, 0:2], g[:, 0:2])     # G_u, G_q
            nc.scalar.copy(outrow[:, 2:4], ts2[:, 0:2])   # t, s
            for j, pname in enumerate(("pc1", "pm1", "pc2", "pm2")):
                nc.vector.tensor_reduce(
                    outrow[:, 4 + j : 5 + j], psums[pname][:, :],
                    axis=AX.X, op=ALU.add,
                )
            nc.sync.dma_start(out_d[:], outrow[:])

            if loop_n:
                pa = LOOP_PARTS_A if LOOP_PARTS_A is not None else PARTS_A
                pb = LOOP_PARTS_B if LOOP_PARTS_B is not None else PARTS_B
                with tc.For_i(0, loop_n, 1):
                    _emit_phase_a(nc, ab_d, ublks, ones, psu, psq,
                                  stage_pool, d_pool, scr_pool, parts=pa)
                    if pb:
                        _emit_phase_b(nc, ones, ublks, thr, psums, scr_pool,
                                      parts=pb)

    nc.compile()
    return nc


_NC_CACHE = None


def _get_program():
    global _NC_CACHE
    if _NC_CACHE is None:
        _NC_CACHE = _build_program()
    return _NC_CACHE


def _shard_inputs(input_img: np.ndarray, target_img: np.ndarray):
    a = np.asarray(input_img, dtype=np.float32)
    b = np.asarray(target_img, dtype=np.float32)
    per = a.shape[0] // N_CORES
    in_maps = []
    for i in range(N_CORES):
        sl = slice(i * per, (i + 1) * per)
        ai = np.ascontiguousarray(a[sl]).reshape(P, N_BLK, 1, BLK)
        bi = np.ascontiguousarray(b[sl]).reshape(P, N_BLK, 1, BLK)
        ab = np.concatenate([ai, bi], axis=2).reshape(P, 2 * FD)
        in_maps.append({"ab": np.ascontiguousarray(ab)})
    return in_maps


def _combine(results) -> np.float32:
    # identical on every core: global sums + thresholds
    row0 = results[0]["partials"].reshape(-1).astype(np.float64)
    g_u, g_q, t, s = row0[0], row0[1], row0[2], row0[3]
    c1 = m1 = c2 = m2 = 0.0
    for res in results:
        row = res["partials"].reshape(-1).astype(np.float64)
        c1 += row[4]
        m1 += row[5]
        c2 += row[6]
        m2 += row[7]

    n = float(N_TOTAL)
    mae_loss = g_u / n
    mse_loss = g_q / n

    s1 = m1 - t * (n - c1)   # sum |d| over u >= t
    s2 = m2 - s * (n - c2)   # sum d^2 over u >= r (q >= s)

    mae_thr = s1 / c1 if c1 > 0 else 0.0
    mse_thr = s2 / c2 if c2 > 0 else 0.0

    combined_thr = 0.5 * mae_thr + 0.5 * mse_thr
    combined_non = 0.5 * mae_loss + 0.5 * mse_loss
    total = 0.5 * combined_thr + 0.5 * combined_non
    return np.float32(total)


def kernel(input_img: np.ndarray, target_img: np.ndarray) -> np.ndarray:
    import time as _time

    nc = _get_program()
    in_maps = _shard_inputs(input_img, target_img)
    last_err = None
    for attempt in range(3):
        try:
            res = run_bass_kernel_spmd(nc, in_maps, list(range(N_CORES)))
            return np.asarray(_combine(res.results))
        except Exception as e:  # transient device-unrecoverable states
            last_err = e
            _time.sleep(20 * (attempt + 1))
    raise last_err
